# revision 7
# baseline (speedup 1.0000x reference)
"""Trainium2 Bass kernel for banded local attention (kernel_size=128).

Problem: x[4,4096,512]; q = x@Wq.T+bq, k = x@Wk.T+bk (H=512);
scores = q@k.T masked to |i-j|<128; softmax; out = attn @ x.

Sharding: 8 cores = 4 batches x 2 sequence halves (2048 queries each) with a
128-row halo of keys on each side (2304 local key rows, zero padded at the
global sequence edges). For the h=1 half the sequence is passed REVERSED so
the padded/invalid key region is always local rows [0,128) and the edge mask
is only needed for query block 0 -> all 8 cores run the identical program
(pure SPMD, no collectives). Host un-reverses the h=1 outputs.

Per-core data layout (all fp32, matmuls in float32r = full-rate fp22):
  xT   [512, 2304]  x_halo transposed (d on partitions) - rhs/lhsT for projs
  xrow [2304, 512]  x_halo row-major - rhs ("values") for attn @ x
  wqT/wkT [512,512] weight transposed [d, h] - lhsT for projections
  bq/bk [512]       biases (added via ACT Identity during PSUM->SBUF copy)
  masks [2,128,384] additive band masks (0 / -1e30); slot 1 = edge variant
On chip:
  qT [h,2048] = wqT.T @ xT (+bq)  4 h-tiles; serves as lhsT for scores
  kT [h,2304] = wkT.T @ xT (+bk)  4 h-tiles; serves as rhs for scores
  per 128-query block qb: s[128,384] = qT_blk.T @ kT_window (PSUM),
  s += mask (DVE), rowmax m (DVE), p = exp(s - m) with accumulated row
  sum l (ACT), pT = PE-transpose(p), out = pT.T @ xrow_window, scaled by
  1/l during the PSUM->SBUF copy (ACT, scale AP).
"""
import sys

if "/opt/trn_rl_repo" not in sys.path:
    sys.path.insert(0, "/opt/trn_rl_repo")

import numpy as np

B, S, D, H = 4, 4096, 512, 512
KS = 128
HALF = S // 2            # 2048 queries per core
HALO = KS                # 128
SK = HALF + 2 * HALO     # 2304 local key rows
WIN = 3 * 128            # 384-wide key window per query block
NBLK = HALF // 128       # 16 query blocks
NEG = -1e30
N_CORES = 8

F32 = None  # set after import
_cached = {}


def _build_program():
    import concourse.bass as bass
    import concourse.tile as tile
    import concourse.mybir as mybir
    from concourse import bacc

    f32 = mybir.dt.float32
    f32r = mybir.dt.float32r
    AF = mybir.ActivationFunctionType
    AX = mybir.AxisListType

    nc = bacc.Bacc("TRN2", target_bir_lowering=False, debug=False,
                   num_devices=N_CORES)

    xT_d = nc.dram_tensor("xT", [D, SK], f32r, kind="ExternalInput").ap()
    xrow_d = nc.dram_tensor("xrow", [SK, D], f32r, kind="ExternalInput").ap()
    wqT_d = nc.dram_tensor("wqT", [D, H], f32r, kind="ExternalInput").ap()
    wkT_d = nc.dram_tensor("wkT", [D, H], f32r, kind="ExternalInput").ap()
    bq_d = nc.dram_tensor("bq", [H, 1], f32, kind="ExternalInput").ap()
    bk_d = nc.dram_tensor("bk", [H, 1], f32, kind="ExternalInput").ap()
    masks_d = nc.dram_tensor("masks", [2, 128, WIN], f32,
                             kind="ExternalInput").ap()
    out_d = nc.dram_tensor("out", [HALF, D], f32, kind="ExternalOutput").ap()

    DT = D // 128   # 4 d-tiles
    HT = H // 128   # 4 h-tiles
    JT = SK // 128  # 18 key row tiles

    with tile.TileContext(nc) as tc:
        with (
            tc.tile_pool(name="big", bufs=1) as big,
            tc.tile_pool(name="work", bufs=3) as work,
            tc.tile_pool(name="stat", bufs=4) as stat,
            tc.tile_pool(name="psA", bufs=2, space="PSUM") as psA,
            tc.tile_pool(name="psB", bufs=2, space="PSUM") as psB,
        ):
            # ---- resident inputs ----
            xT = [big.tile([128, SK], f32r, tag=f"xT{t}", name=f"xT{t}") for t in range(DT)]
            for t in range(DT):
                nc.sync.dma_start(xT[t], xT_d[t * 128:(t + 1) * 128, :])
            xrow = [big.tile([128, D], f32r, tag=f"xr{j}", name=f"xr{j}") for j in range(JT)]
            for j in range(JT):
                nc.sync.dma_start(xrow[j], xrow_d[j * 128:(j + 1) * 128, :])
            wq = [big.tile([128, H], f32r, tag=f"wq{t}", name=f"wq{t}") for t in range(DT)]
            wk = [big.tile([128, H], f32r, tag=f"wk{t}", name=f"wk{t}") for t in range(DT)]
            for t in range(DT):
                nc.sync.dma_start(wq[t], wqT_d[t * 128:(t + 1) * 128, :])
                nc.sync.dma_start(wk[t], wkT_d[t * 128:(t + 1) * 128, :])
            bq = [big.tile([128, 1], f32, tag=f"bq{t}", name=f"bq{t}") for t in range(HT)]
            bk = [big.tile([128, 1], f32, tag=f"bk{t}", name=f"bk{t}") for t in range(HT)]
            for t in range(HT):
                nc.sync.dma_start(bq[t], bq_d[t * 128:(t + 1) * 128, :])
                nc.sync.dma_start(bk[t], bk_d[t * 128:(t + 1) * 128, :])
            masks = [big.tile([128, WIN], f32, tag=f"mask{i}", name=f"maskt{i}") for i in range(2)]
            for i in range(2):
                nc.sync.dma_start(masks[i], masks_d[i])

            # ---- projections: qT[h,i] and kT[h,j] ----
            qT = [big.tile([128, HALF], f32r, tag=f"qT{t}", name=f"qT{t}") for t in range(HT)]
            kT = [big.tile([128, SK], f32r, tag=f"kT{t}", name=f"kT{t}") for t in range(HT)]

            def project(dst, w_tiles, bias, n_cols, off=0):
                # dst[ht][h, c] = sum_d w[d, h] * xT[d, c] + bias[h]
                chunks = []
                c0 = 0
                while c0 < n_cols:
                    cw = min(512, n_cols - c0)
                    chunks.append((c0, cw))
                    c0 += cw
                for ht in range(HT):
                    for (c0, cw) in chunks:
                        ps = psA.tile([128, 512], f32, tag="proj")
                        for dt_i in range(DT):
                            nc.tensor.matmul(
                                ps[:, :cw],
                                lhsT=w_tiles[dt_i][:, ht * 128:(ht + 1) * 128],
                                rhs=xT[dt_i][:, off + c0:off + c0 + cw],
                                start=(dt_i == 0),
                                stop=(dt_i == DT - 1),
                            )
                        nc.scalar.activation(
                            dst[ht][:, c0:c0 + cw], ps[:, :cw],
                            AF.Identity, bias=bias[ht], scale=1.0,
                        )

            project(qT, wq, bq, HALF, off=HALO)
            project(kT, wk, bk, SK)

            # ---- identity for PE transpose ----
            ident = big.tile([128, 128], f32, tag="ident")
            from concourse.masks import make_identity
            make_identity(nc, ident)

            # ---- per query-block attention ----
            for qb in range(NBLK):
                j0 = qb * 128  # key window start (local row / kT col)
                mask = masks[1] if qb == 0 else masks[0]

                s_ps = psA.tile([128, WIN], f32, tag="s")
                for ht in range(HT):
                    nc.tensor.matmul(
                        s_ps,
                        lhsT=qT[ht][:, qb * 128:(qb + 1) * 128],
                        rhs=kT[ht][:, j0:j0 + WIN],
                        start=(ht == 0),
                        stop=(ht == HT - 1),
                    )
                # s += mask ; rowmax ; p = exp(s - m), l = rowsum(p)
                s_sb = work.tile([128, WIN], f32, tag="s_sb")
                nc.vector.tensor_add(s_sb, s_ps, mask)
                m = stat.tile([128, 1], f32, tag="m")
                nc.vector.reduce_max(m, s_sb, axis=AX.X)
                negm = stat.tile([128, 1], f32, tag="negm")
                nc.scalar.mul(negm, m, -1.0)
                p_sb = work.tile([128, WIN], f32, tag="p_sb")
                lsum = stat.tile([128, 1], f32, tag="lsum")
                nc.scalar.activation(p_sb, s_sb, AF.Exp, bias=negm,
                                     scale=1.0, accum_out=lsum)
                rinv = stat.tile([128, 1], f32, tag="rinv")
                nc.vector.reciprocal(rinv, lsum)

                # transpose p -> pT (3 x [128,128])
                pT_ps = psB.tile([128, 3, 128], f32, tag="pT")
                for jt in range(3):
                    nc.tensor.transpose(
                        pT_ps[:, jt, :],
                        p_sb[:, jt * 128:(jt + 1) * 128],
                        ident,
                    )
                pT_sb = work.tile([128, 3, 128], f32r, tag="pT_sb")
                nc.vector.tensor_copy(pT_sb, pT_ps)

                # out_blk[i, d] = sum_j p[i, j] * xrow[j, d], scaled by 1/l
                o_ps = psB.tile([128, D], f32, tag="o")
                for jt in range(3):
                    nc.tensor.matmul(
                        o_ps,
                        lhsT=pT_sb[:, jt, :],
                        rhs=xrow[qb + jt],
                        start=(jt == 0),
                        stop=(jt == 2),
                    )
                o_sb = work.tile([128, D], f32, tag="o_sb")
                nc.scalar.activation(o_sb, o_ps, AF.Identity,
                                     bias=0.0, scale=rinv)
                nc.sync.dma_start(out_d[qb * 128:(qb + 1) * 128, :], o_sb)

    nc.compile()
    return nc


def _get_program():
    if "nc" not in _cached:
        _cached["nc"] = _build_program()
    return _cached["nc"]


def _make_masks():
    a = np.arange(128)[:, None]
    y = np.arange(WIN)[None, :]
    band = (y - a >= 1) & (y - a <= 255)
    base = np.where(band, 0.0, NEG).astype(np.float32)
    edge = np.where(band & (y >= 128), 0.0, NEG).astype(np.float32)
    return np.stack([base, edge])


def kernel(x, Wq_w, Wq_b, Wk_w, Wk_b, _trace=False):
    from concourse.bass_utils import run_bass_kernel_spmd

    x = np.ascontiguousarray(np.asarray(x, np.float32))
    wqT = np.ascontiguousarray(np.asarray(Wq_w, np.float32).T)
    wkT = np.ascontiguousarray(np.asarray(Wk_w, np.float32).T)
    bq = np.ascontiguousarray(np.asarray(Wq_b, np.float32))
    bk = np.ascontiguousarray(np.asarray(Wk_b, np.float32))
    masks = _make_masks()

    nc = _get_program()

    in_maps = []
    for core in range(N_CORES):
        b, h = divmod(core, 2)
        x_halo = np.zeros((SK, D), np.float32)
        if h == 0:
            x_halo[HALO:] = x[b, 0:HALF + HALO]
        else:
            x_halo[HALO:] = x[b, S - HALF - HALO:][::-1]
        in_maps.append({
            "xT": np.ascontiguousarray(x_halo.T),
            "xrow": x_halo,
            "wqT": wqT,
            "wkT": wkT,
            "bq": bq.reshape(H, 1),
            "bk": bk.reshape(H, 1),
            "masks": masks,
        })

    res = run_bass_kernel_spmd(nc, in_maps, core_ids=list(range(N_CORES)),
                               trace=_trace)
    _cached["last_result"] = res

    y = np.zeros((B, S, D), np.float32)
    for core in range(N_CORES):
        b, h = divmod(core, 2)
        o = res.results[core]["out"]
        if h == 0:
            y[b, :HALF] = o
        else:
            y[b, HALF:] = o[::-1]
    return y


# revision 9
# speedup vs baseline: 1.0441x; 1.0441x over previous
"""Trainium2 Bass kernel for banded local attention (kernel_size=128).

Problem: x[4,4096,512]; q = x@Wq.T+bq, k = x@Wk.T+bk (H=512);
scores = q@k.T masked to |i-j|<128; softmax; out = attn @ x.

Sharding: 8 cores = 4 batches x 2 sequence halves (2048 queries each) with a
128-row halo of keys on each side (2304 local key rows, zero padded at the
global sequence edges). For the h=1 half the sequence is passed REVERSED so
the padded/invalid key region is always local rows [0,128) and the edge mask
is only needed for query block 0 -> all 8 cores run the identical program
(pure SPMD, no collectives). Host un-reverses the h=1 outputs.

Per-core data layout (all fp32, matmuls in float32r = full-rate fp22):
  xT   [512, 2304]  x_halo transposed (d on partitions) - rhs/lhsT for projs
  xrow [2304, 512]  x_halo row-major - rhs ("values") for attn @ x
  wqT/wkT [512,512] weight transposed [d, h] - lhsT for projections
  bq/bk [512]       biases (added via ACT Identity during PSUM->SBUF copy)
  masks [2,128,384] additive band masks (0 / -1e30); slot 1 = edge variant
On chip:
  qT [h,2048] = wqT.T @ xT (+bq)  4 h-tiles; serves as lhsT for scores
  kT [h,2304] = wkT.T @ xT (+bk)  4 h-tiles; serves as rhs for scores
  per 128-query block qb: s[128,384] = qT_blk.T @ kT_window (PSUM),
  s += mask (DVE), rowmax m (DVE), p = exp(s - m) with accumulated row
  sum l (ACT), pT = PE-transpose(p), out = pT.T @ xrow_window, scaled by
  1/l during the PSUM->SBUF copy (ACT, scale AP).
"""
import sys

if "/opt/trn_rl_repo" not in sys.path:
    sys.path.insert(0, "/opt/trn_rl_repo")

import numpy as np

B, S, D, H = 4, 4096, 512, 512
KS = 128
HALF = S // 2            # 2048 queries per core
HALO = KS                # 128
SK = HALF + 2 * HALO     # 2304 local key rows
WIN = 3 * 128            # 384-wide key window per query block
NBLK = HALF // 128       # 16 query blocks
NEG = -1e30
N_CORES = 8

F32 = None  # set after import
_cached = {}


def _build_program():
    import concourse.bass as bass
    import concourse.tile as tile
    import concourse.mybir as mybir
    from concourse import bacc

    f32 = mybir.dt.float32
    f32r = mybir.dt.float32r
    AF = mybir.ActivationFunctionType
    AX = mybir.AxisListType

    nc = bacc.Bacc("TRN2", target_bir_lowering=False, debug=False,
                   num_devices=N_CORES)

    xT_d = nc.dram_tensor("xT", [D, SK], f32r, kind="ExternalInput").ap()
    xrow_d = nc.dram_tensor("xrow", [SK, D], f32r, kind="ExternalInput").ap()
    wqT_d = nc.dram_tensor("wqT", [D, H], f32r, kind="ExternalInput").ap()
    wkT_d = nc.dram_tensor("wkT", [D, H], f32r, kind="ExternalInput").ap()
    bq_d = nc.dram_tensor("bq", [H, 1], f32, kind="ExternalInput").ap()
    bk_d = nc.dram_tensor("bk", [H, 1], f32, kind="ExternalInput").ap()
    masks_d = nc.dram_tensor("masks", [2, 128, WIN], f32,
                             kind="ExternalInput").ap()
    out_d = nc.dram_tensor("out", [HALF, D], f32, kind="ExternalOutput").ap()

    DT = D // 128   # 4 d-tiles
    HT = H // 128   # 4 h-tiles
    JT = SK // 128  # 18 key row tiles

    with tile.TileContext(nc) as tc:
        with (
            tc.tile_pool(name="big", bufs=1) as big,
            tc.tile_pool(name="work", bufs=3) as work,
            tc.tile_pool(name="stat", bufs=4) as stat,
            tc.tile_pool(name="psA", bufs=2, space="PSUM") as psA,
            tc.tile_pool(name="psB", bufs=2, space="PSUM") as psB,
        ):
            # ---- resident inputs ----
            wq = [big.tile([128, H], f32r, tag=f"wq{t}", name=f"wq{t}") for t in range(DT)]
            wk = [big.tile([128, H], f32r, tag=f"wk{t}", name=f"wk{t}") for t in range(DT)]
            for t in range(DT):
                nc.sync.dma_start(wq[t], wqT_d[t * 128:(t + 1) * 128, :])
                nc.sync.dma_start(wk[t], wkT_d[t * 128:(t + 1) * 128, :])
            bq = [big.tile([128, 1], f32, tag=f"bq{t}", name=f"bq{t}") for t in range(HT)]
            bk = [big.tile([128, 1], f32, tag=f"bk{t}", name=f"bk{t}") for t in range(HT)]
            for t in range(HT):
                nc.sync.dma_start(bq[t], bq_d[t * 128:(t + 1) * 128, :])
                nc.sync.dma_start(bk[t], bk_d[t * 128:(t + 1) * 128, :])
            masks = [big.tile([128, WIN], f32, tag=f"mask{i}", name=f"maskt{i}") for i in range(2)]
            for i in range(2):
                nc.sync.dma_start(masks[i], masks_d[i])
            xT = [big.tile([128, SK], f32r, tag=f"xT{t}", name=f"xT{t}") for t in range(DT)]
            for t in range(DT):
                nc.sync.dma_start(xT[t], xT_d[t * 128:(t + 1) * 128, :])

            # ---- projections: qT[h,i] and kT[h,j] ----
            qT = [big.tile([128, HALF], f32r, tag=f"qT{t}", name=f"qT{t}") for t in range(HT)]
            kT = [big.tile([128, SK], f32r, tag=f"kT{t}", name=f"kT{t}") for t in range(HT)]

            def project(dst, w_tiles, bias, n_cols, off=0, on_dve=False):
                # dst[ht][h, c] = sum_d w[d, h] * xT[d, c] + bias[h]
                chunks = []
                c0 = 0
                while c0 < n_cols:
                    cw = min(512, n_cols - c0)
                    chunks.append((c0, cw))
                    c0 += cw
                for ht in range(HT):
                    for (c0, cw) in chunks:
                        ps = psA.tile([128, 512], f32, tag="proj")
                        for dt_i in range(DT):
                            nc.tensor.matmul(
                                ps[:, :cw],
                                lhsT=w_tiles[dt_i][:, ht * 128:(ht + 1) * 128],
                                rhs=xT[dt_i][:, off + c0:off + c0 + cw],
                                start=(dt_i == 0),
                                stop=(dt_i == DT - 1),
                            )
                        if on_dve:
                            nc.vector.tensor_scalar_add(
                                dst[ht][:, c0:c0 + cw], ps[:, :cw], bias[ht])
                        else:
                            nc.scalar.activation(
                                dst[ht][:, c0:c0 + cw], ps[:, :cw],
                                AF.Identity, bias=bias[ht], scale=1.0,
                            )

            project(qT, wq, bq, HALF, off=HALO, on_dve=True)
            project(kT, wk, bk, SK)

            # xrow ("values") DMAs issued after the projection instructions so
            # the Sync queue services weights/xT first; these 4.7MB stream in
            # while the PE is busy with the projections.
            xrow = [big.tile([128, D], f32r, tag=f"xr{j}", name=f"xr{j}")
                    for j in range(JT)]
            for j in range(JT):
                nc.sync.dma_start(xrow[j], xrow_d[j * 128:(j + 1) * 128, :])

            # ---- identity for PE transpose ----
            ident = big.tile([128, 128], f32, tag="ident")
            from concourse.masks import make_identity
            make_identity(nc, ident)

            # ---- per query-block attention ----
            for qb in range(NBLK):
                j0 = qb * 128  # key window start (local row / kT col)
                mask = masks[1] if qb == 0 else masks[0]

                s_ps = psA.tile([128, WIN], f32, tag="s")
                for ht in range(HT):
                    nc.tensor.matmul(
                        s_ps,
                        lhsT=qT[ht][:, qb * 128:(qb + 1) * 128],
                        rhs=kT[ht][:, j0:j0 + WIN],
                        start=(ht == 0),
                        stop=(ht == HT - 1),
                    )
                # s += mask ; rowmax ; p = exp(s - m), l = rowsum(p)
                s_sb = work.tile([128, WIN], f32, tag="s_sb")
                nc.vector.tensor_add(s_sb, s_ps, mask)
                m = stat.tile([128, 1], f32, tag="m")
                nc.vector.reduce_max(m, s_sb, axis=AX.X)
                negm = stat.tile([128, 1], f32, tag="negm")
                nc.scalar.mul(negm, m, -1.0)
                p_sb = work.tile([128, WIN], f32, tag="p_sb")
                lsum = stat.tile([128, 1], f32, tag="lsum")
                nc.scalar.activation(p_sb, s_sb, AF.Exp, bias=negm,
                                     scale=1.0, accum_out=lsum)
                rinv = stat.tile([128, 1], f32, tag="rinv")
                nc.vector.reciprocal(rinv, lsum)

                # transpose p -> pT (3 x [128,128])
                pT_ps = psB.tile([128, 3, 128], f32, tag="pT")
                for jt in range(3):
                    nc.tensor.transpose(
                        pT_ps[:, jt, :],
                        p_sb[:, jt * 128:(jt + 1) * 128],
                        ident,
                    )
                pT_sb = work.tile([128, 3, 128], f32r, tag="pT_sb")
                nc.vector.tensor_copy(pT_sb, pT_ps)

                # out_blk[i, d] = sum_j p[i, j] * xrow[j, d], scaled by 1/l
                o_ps = psB.tile([128, D], f32, tag="o")
                for jt in range(3):
                    nc.tensor.matmul(
                        o_ps,
                        lhsT=pT_sb[:, jt, :],
                        rhs=xrow[qb + jt],
                        start=(jt == 0),
                        stop=(jt == 2),
                    )
                o_sb = work.tile([128, D], f32, tag="o_sb")
                nc.scalar.activation(o_sb, o_ps, AF.Identity,
                                     bias=0.0, scale=rinv)
                nc.sync.dma_start(out_d[qb * 128:(qb + 1) * 128, :], o_sb)

    nc.compile()
    return nc


def _get_program():
    if "nc" not in _cached:
        _cached["nc"] = _build_program()
    return _cached["nc"]


def _make_masks():
    a = np.arange(128)[:, None]
    y = np.arange(WIN)[None, :]
    band = (y - a >= 1) & (y - a <= 255)
    base = np.where(band, 0.0, NEG).astype(np.float32)
    edge = np.where(band & (y >= 128), 0.0, NEG).astype(np.float32)
    return np.stack([base, edge])


def kernel(x, Wq_w, Wq_b, Wk_w, Wk_b, _trace=False):
    from concourse.bass_utils import run_bass_kernel_spmd

    x = np.ascontiguousarray(np.asarray(x, np.float32))
    wqT = np.ascontiguousarray(np.asarray(Wq_w, np.float32).T)
    wkT = np.ascontiguousarray(np.asarray(Wk_w, np.float32).T)
    bq = np.ascontiguousarray(np.asarray(Wq_b, np.float32))
    bk = np.ascontiguousarray(np.asarray(Wk_b, np.float32))
    masks = _make_masks()

    nc = _get_program()

    in_maps = []
    for core in range(N_CORES):
        b, h = divmod(core, 2)
        x_halo = np.zeros((SK, D), np.float32)
        if h == 0:
            x_halo[HALO:] = x[b, 0:HALF + HALO]
        else:
            x_halo[HALO:] = x[b, S - HALF - HALO:][::-1]
        in_maps.append({
            "xT": np.ascontiguousarray(x_halo.T),
            "xrow": x_halo,
            "wqT": wqT,
            "wkT": wkT,
            "bq": bq.reshape(H, 1),
            "bk": bk.reshape(H, 1),
            "masks": masks,
        })

    res = run_bass_kernel_spmd(nc, in_maps, core_ids=list(range(N_CORES)),
                               trace=_trace)
    _cached["last_result"] = res

    y = np.zeros((B, S, D), np.float32)
    for core in range(N_CORES):
        b, h = divmod(core, 2)
        o = res.results[core]["out"]
        if h == 0:
            y[b, :HALF] = o
        else:
            y[b, HALF:] = o[::-1]
    return y
